# revision 14
# baseline (speedup 1.0000x reference)
"""Trainium2 Bass kernel for nn_DenoiseNet (dense_cnn), 8-core data parallel.

Algorithm (exact, validated vs reference in fp32):
  The kernel-prediction path (conv3x3(3->64) -> depthwise3x3 -> 1x1(64->27))
  is linear in x, so it is folded on the host into ONE composed 5x5 conv
  (3->27 channels) plus:
    - a spatially-uniform term from the ECA channel attention, computed on
      device from cheap reductions of x via a host-folded 27x73 linear map,
    - 1-pixel border-ring corrections computed on device as small matmuls.
  Dynamic filtering = elementwise multiply of the 27 per-pixel kernel maps with
  9 shifted copies of x, then a 0/1 tap-reduction matmul.

Performance structure (vs the previous revision):
  - x is loaded once as fp32 into SBUF; the padded bf16 DRAM staging (xpg) is
    produced by gpsimd cast-DMAs (SWDGE path: no HWDGE contention, casts in
    flight, 3-dim APs).
  - Per band only 5 gpsimd DMAs build the im2col (2) and tap-shift (3) tiles;
    output-channel order is host-permuted to (ty, g, j, tx) so every gather is
    a contiguous/strided partition slice with a <=3-dim AP.
  - Main conv accumulates row PAIRS into 2-bank PSUM tiles; single ACT
    activation drains each pair with the uniform-term bias.
  - Tap matmuls of band b are issued after the main matmuls of band b+1
    (software pipelining) so the PE stream never waits on the DVE multiply;
    PSUM out tiles are drained by DVE and stored by SP.
"""
import numpy as np
import ml_dtypes

import concourse.bass as bass
import concourse.bacc as bacc
import concourse.mybir as mybir
import concourse.tile as tile
from concourse.ap import AP
from concourse.bass_utils import run_bass_kernel_spmd

BF16 = ml_dtypes.bfloat16
H = W = 512
HP, WP = 516, 520
CH = 64
KO = 27
N_CORES = 8
BANDS = 8
BR = 16          # rows per g-chunk per band
CHS = HP * WP
GRS = 128 * WP
BS = 133 * WP    # xpg block stride

F32 = mybir.dt.float32
BF = mybir.dt.bfloat16

XFREE = 4 * 3 * 524  # xf32 free size per partition (elements)


# ------------------------------------------------------------------ host math
def _conv_compose_2d(wa, wb):
    """Compose two cross-correlation kernels (wb after wa).
    wa [C, I, 3, 3], wb [C, 3, 3] depthwise -> [C, I, 5, 5]."""
    C, I = wa.shape[:2]
    out = np.zeros((C, I, 5, 5), wa.dtype)
    for a in range(3):
        for b in range(3):
            out[:, :, a:a + 3, b:b + 3] += wb[:, a, b][:, None, None, None] * wa
    return out


def precompute(w_feat, b_feat, w_sa, b_sa, w_ca, b_ca, w_k, b_k):
    """Fold all weights. float64 internally. Returns dict of np arrays."""
    w_feat = w_feat.astype(np.float64)
    b_feat = b_feat.astype(np.float64)
    w_sa0 = w_sa[:, 0].astype(np.float64)
    b_sa = b_sa.astype(np.float64)
    w_ca = np.asarray(w_ca).astype(np.float64)
    b_ca = float(np.asarray(b_ca).reshape(-1)[0])
    w_k = w_k.astype(np.float64)
    b_k = b_k.astype(np.float64)

    # composed 5x5
    W5 = _conv_compose_2d(w_feat, w_sa0)                 # [C, 3, 5, 5]
    WK5 = np.einsum("oc,cist->oist", w_k, W5)            # [27, 3, 5, 5]
    W1 = w_sa0.sum(axis=(1, 2))
    const_o = w_k @ (b_feat * W1 + b_sa) + b_k           # [27]

    # w5e[pp][k=30g+10i+2dy+dx, m=27g+o] (old m-order; permuted at the end)
    w5e = np.zeros((3, 120, 108))
    for pp in range(3):
        for g in range(4):
            for i in range(3):
                for dy in range(5):
                    for dx in range(2):
                        dxa = 2 * pp + dx
                        if dxa > 4:
                            continue
                        k = 30 * g + 10 * i + 2 * dy + dx
                        w5e[pp, k, 27 * g + np.arange(27)] = WK5[:, i, dy, dxa]
    # sel [108, 12]: row 27g+9j+t -> col 3g+j
    sel = np.zeros((108, 12))
    for g in range(4):
        for j in range(3):
            for t in range(9):
                sel[27 * g + 9 * j + t, 3 * g + j] = 1.0

    # ---- cvec = BV.T @ v (old v layout, re-rowed at the end):
    #  0..11  P1[(s,i)]  12..23 P2L  24..35 P2R  36..47 RS0  48..59 RSL
    #  60..71 corners x[i, 511cy, 511cx] at 60+cy*6+i*2+cx; 72 = 1.0
    MxT = np.zeros((3, 3, 3, 73))
    for i in range(3):
        S = np.zeros(73)
        for s in range(4):
            S[s * 3 + i] = 1.0
        r_ex = {0: np.zeros(73), 1: None, 2: np.zeros(73)}
        r_ex[0][48 + 9 + i] = 1.0     # RSL s=3 -> row 511 excluded for a=0
        r_ex[2][36 + 0 + i] = 1.0     # RS0 s=0 -> row 0 excluded for a=2
        c_ex = {0: np.zeros(73), 1: None, 2: np.zeros(73)}
        for s in range(4):
            c_ex[0][24 + s * 3 + i] = 1.0   # P2R: col 511
            c_ex[2][12 + s * 3 + i] = 1.0   # P2L: col 0
        corner = {(0, 0): (1, 1), (0, 2): (1, 0), (2, 0): (0, 1), (2, 2): (0, 0)}
        for a in range(3):
            for b in range(3):
                m = S.copy()
                if r_ex[a] is not None:
                    m -= r_ex[a]
                if c_ex[b] is not None:
                    m -= c_ex[b]
                if (a, b) in corner:
                    cy, cx = corner[(a, b)]
                    m[60 + cy * 6 + i * 2 + cx] += 1.0
                MxT[i, a, b] = m
    meanT = np.einsum("ciab,iabv->cv", w_feat, MxT) / (H * W)   # [C, 73]
    meanT[:, 72] += b_feat
    caT = np.zeros((CH, 73))
    for d in range(3):
        lo = max(0, 1 - d)
        hi = min(CH, CH + 1 - d)
        caT[lo:hi] += w_ca[0, 0, d] * meanT[d - 1 + lo: d - 1 + hi]
    caT[:, 72] += b_ca
    cvT = w_k @ caT
    cvT[:, 72] += const_o
    BV = np.zeros((73, 108))
    for g in range(4):
        BV[:, 27 * g:27 * g + 27] = cvT.T

    # ---- borders ----
    def wb_1d(sa_row, feat_row):
        out = np.zeros((KO, 3, 5))
        for qx in range(3):
            wk_sa = w_k * sa_row[:, qx][None, :]
            for dx in range(3):
                out[:, :, qx + dx] += wk_sa @ feat_row[:, :, dx]
        return out

    WBtop = wb_1d(w_sa0[:, 0, :], w_feat[:, :, 2, :])
    WBbot = wb_1d(w_sa0[:, 2, :], w_feat[:, :, 0, :])
    WBleft = wb_1d(w_sa0[:, :, 0], w_feat[:, :, :, 2])
    WBright = wb_1d(w_sa0[:, :, 2], w_feat[:, :, :, 0])
    cW = {"t": w_k @ (w_sa0[:, 0, :].sum(1) * b_feat),
          "b": w_k @ (w_sa0[:, 2, :].sum(1) * b_feat),
          "l": w_k @ (w_sa0[:, :, 0].sum(1) * b_feat),
          "r": w_k @ (w_sa0[:, :, 2].sum(1) * b_feat)}

    tw = np.zeros((31, 108))
    bw = np.zeros((31, 108))
    for i in range(3):
        for s in range(5):
            tw[i * 5 + s, 0:27] = WBtop[:, i, s]
            bw[15 + i * 5 + s, 81:108] = WBbot[:, i, s]
    tw[30, 0:27] = cW["t"]
    bw[30, 81:108] = cW["b"]

    # lr weights: row q = 20i + 5g + s (strip-gather order), row 60 = ones
    lw = np.zeros((61, 108))
    rw = np.zeros((61, 108))
    for g in range(4):
        for i in range(3):
            for s in range(5):
                q = 20 * i + 5 * g + s
                lw[q, 27 * g:27 * g + 27] = WBleft[:, i, s]
                rw[q, 27 * g:27 * g + 27] = WBright[:, i, s]
        lw[60, 27 * g:27 * g + 27] = cW["l"]
        rw[60, 27 * g:27 * g + 27] = cW["r"]

    # corner double-count add-backs, split into top (g-block 0) and bottom
    # (g-block 3) variants so edge bands can apply full-width vectors.
    cwl_t = np.zeros((7, 108))
    cwl_b = np.zeros((7, 108))
    cwr_t = np.zeros((7, 108))
    cwr_b = np.zeros((7, 108))
    specs = [
        (cwl_t, 0, 0, (0, 0), (2, 2)),   # TL
        (cwl_b, 1, 3, (2, 0), (0, 2)),   # BL
        (cwr_t, 0, 0, (0, 2), (2, 0)),   # TR
        (cwr_b, 1, 3, (2, 2), (0, 0)),   # BR
    ]
    for M, cy, gblk, (qy, qx), (a, b) in specs:
        wk_sa = w_k * w_sa0[:, qy, qx][None, :]
        A = wk_sa @ w_feat[:, :, a, b]
        c0 = wk_sa @ b_feat
        for i in range(3):
            M[i * 2 + cy, 27 * gblk:27 * gblk + 27] = A[:, i]
        M[6, 27 * gblk:27 * gblk + 27] += c0

    # ---- output-channel permutation: m_new = 36ty + 9g + 3j + tx ----
    perm = np.zeros(108, np.int64)
    for g in range(4):
        for j in range(3):
            for ty in range(3):
                for tx in range(3):
                    m_old = 27 * g + 9 * j + 3 * ty + tx
                    m_new = 36 * ty + 9 * g + 3 * j + tx
                    perm[m_new] = m_old
    w5e = w5e[:, :, perm]
    sel = sel[perm, :]
    tw = -tw[:, perm]
    bw = -bw[:, perm]
    lw = -lw[:, perm]
    rw = -rw[:, perm]
    cwl_t = cwl_t[:, perm]
    cwl_b = cwl_b[:, perm]
    cwr_t = cwr_t[:, perm]
    cwr_b = cwr_b[:, perm]
    BV = BV[:, perm]
    # v-row reorder: first 36 entries interleaved as v_new[3*si + kind]
    BVr = BV.copy()
    for si in range(12):
        for kind in range(3):
            BVr[3 * si + kind] = BV[kind * 12 + si]
    return dict(w5e=w5e, sel=sel, BV=BVr, tw=tw, bw=bw, lw=lw, rw=rw,
                cwl_t=cwl_t, cwl_b=cwl_b, cwr_t=cwr_t, cwr_b=cwr_b,
                eye=np.eye(128))


def make_wmaps(pre):
    """Device weight arrays with final dtypes (BV fp32, rest bf16)."""
    out = {}
    for k, v in pre.items():
        if k == "BV":
            out[k] = np.ascontiguousarray(v.astype(np.float32))
        else:
            out[k] = np.ascontiguousarray(v.astype(np.float32).astype(BF16))
    return out


# ------------------------------------------------------------------ device IR
def _h(t):
    """TensorHandle from handle-or-AP."""
    return getattr(t, "tensor", t)


def build_nc():
    nc = bacc.Bacc("TRN2", target_bir_lowering=False, debug=False,
                   num_devices=N_CORES)
    x_ext = nc.declare_dram_parameter("x", (3, H, W), F32, isOutput=False)
    out_ext = nc.declare_dram_parameter("out", (3, H, W), F32, isOutput=True)
    wnames = {"w5e": ((3, 120, 108), BF), "sel": ((108, 12), BF),
              "BV": ((73, 108), F32),
              "tw": ((31, 108), BF), "bw": ((31, 108), BF),
              "lw": ((61, 108), BF), "rw": ((61, 108), BF),
              "cwl_t": ((7, 108), BF), "cwl_b": ((7, 108), BF),
              "cwr_t": ((7, 108), BF), "cwr_b": ((7, 108), BF),
              "eye": ((128, 128), BF)}
    wext = {k: nc.declare_dram_parameter(k, shp, dt, isOutput=False)
            for k, (shp, dt) in wnames.items()}
    xpg = nc.dram_tensor("xpg", (12, 133, WP), BF)
    strip = nc.dram_tensor("strip", (3, 2, 516), F32)

    IDENT = mybir.ActivationFunctionType.Identity
    SUB = mybir.AluOpType.subtract
    ADD = mybir.AluOpType.add
    MULT = mybir.AluOpType.mult

    _ring = [0]

    def dma(out, in_):
        """Alternate small prologue DMAs across the two HWDGE engines."""
        _ring[0] ^= 1
        eng = nc.sync if _ring[0] else nc.scalar
        eng.dma_start(out, in_)

    with tile.TileContext(nc) as tc:
        with tc.tile_pool(name="const", bufs=1) as cpool, \
             tc.tile_pool(name="xres", bufs=1) as xpool, \
             tc.tile_pool(name="psPro", bufs=2, space="PSUM") as psP:

            # ---- x loads first (long pole on HWDGE/DMA engines) ----
            xf = xpool.tile([128, 4, 3, 524], F32)
            pitch = xf[:].ap[0][0]
            xft = _h(xf[:].tensor)
            xoff = xf[:].offset
            nc.vector.memset(xf[:, :, :, 0:2], 0.0)
            nc.vector.memset(xf[:, :, :, 514:524], 0.0)
            for hh in range(2):
                p0 = 64 * hh
                for i in range(3):
                    eng = nc.sync if (i + hh) % 2 else nc.scalar
                    eng.dma_start(
                        xf[p0:p0 + 64, :, i, 2:514],
                        AP(_h(x_ext), i * 262144 + p0 * 512,
                           [[512, 64], [65536, 4], [1, 512]]))

            # ---- weights ----
            w5e_sb = cpool.tile([120, 3, 108], BF)
            dma(w5e_sb[:], wext["w5e"][:].transpose([1, 0, 2]))
            sel_sb = cpool.tile([108, 12], BF)
            dma(sel_sb[:], wext["sel"][:])
            bv_sb = cpool.tile([73, 108], F32)
            dma(bv_sb[:], wext["BV"][:])
            tw_sb = cpool.tile([31, 108], BF)
            dma(tw_sb[:], wext["tw"][:])
            bw_sb = cpool.tile([31, 108], BF)
            dma(bw_sb[:], wext["bw"][:])
            lw_sb = cpool.tile([61, 108], BF)
            dma(lw_sb[:], wext["lw"][:])
            rw_sb = cpool.tile([61, 108], BF)
            dma(rw_sb[:], wext["rw"][:])
            cw_sb = {}
            for k in ("cwl_t", "cwl_b", "cwr_t", "cwr_b"):
                cw_sb[k] = cpool.tile([7, 108], BF, name=k + "_w")
                dma(cw_sb[k][:], wext[k][:])
            eye_sb = cpool.tile([128, 128], BF)
            dma(eye_sb[:], wext["eye"][:])

            ztile = cpool.tile([12, 1056], BF)
            nc.vector.memset(ztile[:], 0.0)
            ztf = cpool.tile([3, 24], F32)
            nc.vector.memset(ztf[:], 0.0)
            onescol = cpool.tile([128, 1], F32)
            nc.vector.memset(onescol[:], 1.0)
            onesbf = cpool.tile([1, 512], BF)
            nc.vector.memset(onesbf[:], 1.0)

            # ---- stage xpg (gpsimd cast-DMAs) ----
            for hh in range(2):
                p0 = 64 * hh
                nc.gpsimd.dma_start(
                    AP(_h(xpg), (2 + p0) * WP + 2, [[WP, 64], [BS, 12], [1, 512]]),
                    AP(xft, xoff + p0 * pitch + 2, [[pitch, 64], [524, 12], [1, 512]]))
            # halos: bottom halo of blocks 0..8 <- next chunk rows 0:2
            nc.gpsimd.dma_start(
                AP(_h(xpg), 130 * WP + 2, [[WP, 2], [BS, 9], [1, 512]]),
                AP(xft, xoff + 3 * 524 + 2, [[pitch, 2], [524, 9], [1, 512]]))
            # top halo of blocks 3..11 <- prev chunk rows 126:128
            nc.gpsimd.dma_start(
                AP(_h(xpg), 3 * BS + 2, [[WP, 2], [BS, 9], [1, 512]]),
                AP(xft, xoff + 126 * pitch + 2, [[pitch, 2], [524, 9], [1, 512]]))
            # zero pads
            dma(AP(_h(xpg), 0, [[BS, 3], [1, 2 * WP]]), ztile[0:3, 0:2 * WP])
            dma(AP(_h(xpg), 9 * BS + 130 * WP, [[BS, 3], [1, 2 * WP]]),
                ztile[0:3, 0:2 * WP])
            dma(AP(_h(xpg), 0, [[BS, 12], [WP, 132], [1, 2]]),
                ztile[0:12, 0:264])
            dma(AP(_h(xpg), 514, [[BS, 12], [WP, 132], [1, 6]]),
                ztile[0:12, 0:792])

            # ---- column strips for L/R border maps ----
            for i in range(3):
                for side, col in ((0, 2), (1, 513)):
                    dma(AP(_h(strip), (2 * i + side) * 516 + 2,
                           [[1, 128], [128, 4], [1, 1]]),
                        AP(xft, xoff + i * 524 + col,
                           [[pitch, 128], [1572, 4], [1, 1]]))
            dma(AP(_h(strip), 0, [[516, 6], [1, 2]]), ztf[0:3, 0:4])
            dma(AP(_h(strip), 514, [[516, 6], [1, 2]]), ztf[0:3, 0:4])

            # ---- reductions for the uniform (ca) term ----
            rs = cpool.tile([128, 12], F32)
            nc.vector.reduce_sum(rs[:], xf[:, :, :, 2:514],
                                 axis=mybir.AxisListType.X)
            pvt = psP.tile([128, 512], F32, tag="pro", bufs=2, name="pv_ps")
            nc.tensor.matmul(pvt[0:12, 0:1], lhsT=rs[:], rhs=onescol[:],
                             start=True, stop=True)
            nc.tensor.matmul(pvt[0:12, 1:2], lhsT=xf[:, :, :, 2],
                             rhs=onescol[:], start=True, stop=True)
            nc.tensor.matmul(pvt[0:12, 2:3], lhsT=xf[:, :, :, 513],
                             rhs=onescol[:], start=True, stop=True)
            pv_sb = cpool.tile([12, 3], F32)
            nc.scalar.activation(pv_sb[:], pvt[0:12, 0:3], IDENT)

            vt = cpool.tile([73, 1], F32)
            dma(vt[0:36], pv_sb[:])
            dma(vt[36:48], rs[0:1, :])
            dma(vt[48:60], rs[127:128, :])
            for cy in range(2):
                for cx in range(2):
                    dma(vt[60 + cy * 6 + cx:66 + cy * 6 + cx:2],
                        AP(xft, xoff + cy * 127 * pitch + (9 * cy) * 524 + 2
                           + cx * 511,
                           [[pitch, 1], [524, 3], [1, 1]]))
            dma(vt[72:73], onescol[0:1, 0:1])
            cvp = psP.tile([128, 512], F32, tag="pro", bufs=2, name="cv_ps")
            nc.tensor.matmul(cvp[0:108, 0:1], lhsT=bv_sb[:], rhs=vt[:],
                             start=True, stop=True)
            cvec_sb = cpool.tile([108, 1], F32)
            nc.scalar.activation(cvec_sb[:], cvp[0:108, 0:1], IDENT)

            # ---- border-map rhs gathers ----
            tb_rhs = cpool.tile([31, 512], BF)
            dma(tb_rhs[0:15], AP(_h(xpg), 2 * WP, [[BS, 3], [1, 5], [1, 512]]))
            dma(tb_rhs[15:30],
                AP(_h(xpg), 9 * BS + 129 * WP, [[BS, 3], [1, 5], [1, 512]]))
            dma(tb_rhs[30:31], onesbf[0:1, 0:512])
            lr_rhsL = cpool.tile([61, 128], BF)
            lr_rhsR = cpool.tile([61, 128], BF)
            for i in range(3):
                nc.gpsimd.dma_start(
                    lr_rhsL[20 * i:20 * i + 20],
                    AP(_h(strip), (2 * i + 0) * 516,
                       [[128, 4], [1, 5], [1, 128]]))
                nc.gpsimd.dma_start(
                    lr_rhsR[20 * i:20 * i + 20],
                    AP(_h(strip), (2 * i + 1) * 516,
                       [[128, 4], [1, 5], [1, 128]]))
            dma(lr_rhsL[60:61], onesbf[0:1, 0:128])
            dma(lr_rhsR[60:61], onesbf[0:1, 0:128])
            crl = cpool.tile([7, 1], BF)
            dma(crl[0:6], AP(_h(xpg), 2 * WP + 2,
                             [[BS, 3], [9 * BS + 127 * WP, 2], [1, 1]]))
            dma(crl[6:7], onesbf[0:1, 0:1])
            crr = cpool.tile([7, 1], BF)
            dma(crr[0:6], AP(_h(xpg), 2 * WP + 513,
                             [[BS, 3], [9 * BS + 127 * WP, 2], [1, 1]]))
            dma(crr[6:7], onesbf[0:1, 0:1])

            # ---- transposed border maps (accumulated into kps later) ----
            def mm_to_bf(name, lhsT, rhs, pdim, ndim):
                ps = psP.tile([128, 512], F32, tag="pro", bufs=2,
                              name=name + "_ps")
                nc.tensor.matmul(ps[0:pdim, 0:ndim], lhsT=lhsT, rhs=rhs,
                                 start=True, stop=True)
                sb = cpool.tile([pdim, ndim], BF, name=name)
                nc.scalar.activation(sb[:], ps[0:pdim, 0:ndim], IDENT)
                return sb

            # lrLT[row, m] = -sum_q lr_rhsL[q, row] * lw[q, m]
            lrLT_sb = mm_to_bf("lrLT_sb", lr_rhsL[:], lw_sb[:], 128, 108)
            lrRT_sb = mm_to_bf("lrRT_sb", lr_rhsR[:], rw_sb[:], 128, 108)
            c4 = {k: mm_to_bf(k + "T", (crl if "l_" in k else crr)[:],
                              cw_sb[k][:], 1, 108)
                  for k in ("cwl_t", "cwl_b", "cwr_t", "cwr_b")}

            # ---- band loop, software-pipelined taps ----
            bp_ctx = tc.tile_pool(name="band", bufs=1)
            bpool = bp_ctx.__enter__()
            psK_ctx = tc.tile_pool(name="psK", bufs=1, space="PSUM")
            psK = psK_ctx.__enter__()
            psO_ctx = tc.tile_pool(name="psO", bufs=1, space="PSUM")
            psO = psO_ctx.__enter__()

            def emit_taps(pb, prodp):
                r0p = pb * BR
                outb = bpool.tile([76, 8, 512], F32, tag="outsb", bufs=2,
                                  name=f"outsb{pb}")
                for q in range(8):
                    o2 = psO.tile([76, 512], F32, tag="o2", bufs=2,
                                  name=f"o2_{pb}_{q}")
                    nc.tensor.matmul(o2[0:12, :], lhsT=sel_sb[:],
                                     rhs=prodp[:, 2 * q, :],
                                     start=True, stop=True)
                    nc.tensor.matmul(o2[64:76, :], lhsT=sel_sb[:],
                                     rhs=prodp[:, 2 * q + 1, :],
                                     start=True, stop=True)
                    nc.scalar.activation(outb[:, q, :], o2[:], IDENT)
                for j in range(3):
                    nc.sync.dma_start(
                        AP(_h(out_ext), j * 262144 + r0p * 512,
                           [[65536, 4], [1024, 8], [1, 512]]),
                        outb[j:12:3])
                    nc.sync.dma_start(
                        AP(_h(out_ext), j * 262144 + (r0p + 1) * 512,
                           [[65536, 4], [1024, 8], [1, 512]]),
                        outb[64 + j:76:3])

            order = [1, 2, 3, 4, 5, 6, 0, 7]

            def emit_loads(b):
                r0 = b * BR
                imcol = bpool.tile([120, BR, 520], BF, tag="imcol", bufs=3,
                                   name=f"imcol{b}")
                for dx in range(2):
                    eng = nc.sync if dx == 0 else nc.scalar
                    eng.dma_start(
                        imcol[dx::2],
                        AP(_h(xpg), r0 * WP + dx,
                           [[BS, 12], [WP, 5], [1, BR * WP]]))
                xs = bpool.tile([108, BR, 520], BF, tag="xs", bufs=3,
                                name=f"xs{b}")
                for ty in range(3):
                    nc.gpsimd.dma_start(
                        xs[36 * ty:36 * ty + 36],
                        AP(_h(xpg), (r0 + ty + 1) * WP + 1,
                           [[BS, 12], [1, 3], [1, BR * WP]]))
                return imcol, xs

            loads = {order[0]: emit_loads(order[0])}
            prev = None
            for idx, b in enumerate(order):
                r0 = b * BR
                imcol, xs = loads.pop(b)
                if idx + 1 < len(order):
                    loads[order[idx + 1]] = emit_loads(order[idx + 1])

                ks = bpool.tile([108, BR, 512], BF, tag="ks", bufs=2,
                                name=f"ks{b}")
                for pr in range(8):
                    kps = psK.tile([108, 2, 512], F32, tag="kps", bufs=2,
                                   name=f"kps{b}_{pr}")
                    for rr in range(2):
                        r = 2 * pr + rr
                        R = r0 + r
                        chain = [(kps[:, rr, :], w5e_sb[:, 0, :],
                                  imcol[:, r, 0:512]),
                                 (kps[:, rr, :], w5e_sb[:, 1, :],
                                  imcol[:, r, 2:514]),
                                 (kps[:, rr, :], w5e_sb[:, 2, :],
                                  imcol[:, r, 4:516])]
                        if b == 0 and r == 0:
                            chain.append((kps[:, rr, :], tw_sb[:], tb_rhs[:]))
                            chain.append((kps[:, rr, 0:1], c4["cwl_t"][:],
                                          onesbf[0:1, 0:1]))
                            chain.append((kps[:, rr, 511:512], c4["cwr_t"][:],
                                          onesbf[0:1, 0:1]))
                        if b == BANDS - 1 and r == BR - 1:
                            chain.append((kps[:, rr, :], bw_sb[:], tb_rhs[:]))
                            chain.append((kps[:, rr, 0:1], c4["cwl_b"][:],
                                          onesbf[0:1, 0:1]))
                            chain.append((kps[:, rr, 511:512], c4["cwr_b"][:],
                                          onesbf[0:1, 0:1]))
                        chain.append((kps[:, rr, 0:1], lrLT_sb[:],
                                      eye_sb[:, R:R + 1]))
                        chain.append((kps[:, rr, 511:512], lrRT_sb[:],
                                      eye_sb[:, R:R + 1]))
                        for ci, (dst, lh, rh) in enumerate(chain):
                            nc.tensor.matmul(dst, lhsT=lh, rhs=rh,
                                             start=(ci == 0),
                                             stop=(ci == len(chain) - 1))
                    sl = slice(2 * pr, 2 * pr + 2)
                    nc.scalar.activation(ks[:, sl, :], kps[:], IDENT,
                                         bias=cvec_sb[:])
                    nc.vector.tensor_tensor(ks[:, sl, :], ks[:, sl, :],
                                            xs[:, sl, 0:512], op=MULT)
                if prev is not None:
                    emit_taps(*prev)
                prev = (b, ks)
            emit_taps(*prev)
            psO_ctx.__exit__(None, None, None)
            psK_ctx.__exit__(None, None, None)
            bp_ctx.__exit__(None, None, None)
    nc.compile()
    return nc


_NC_CACHE = None


def prepare_in_maps(inputs):
    x = np.ascontiguousarray(np.asarray(inputs["x"], np.float32))
    pre = precompute(inputs["w_feat"], inputs["b_feat"], inputs["w_sa"],
                     inputs["b_sa"], inputs["w_ca"], inputs["b_ca"],
                     inputs["w_k"], inputs["b_k"])
    wmaps = make_wmaps(pre)
    return [dict(x=x[c], **wmaps) for c in range(N_CORES)]


def kernel(**inputs):
    global _NC_CACHE
    if _NC_CACHE is None:
        _NC_CACHE = build_nc()
    nc = _NC_CACHE
    in_maps = prepare_in_maps(inputs)
    res = run_bass_kernel_spmd(nc, in_maps, core_ids=list(range(N_CORES)))
    out = np.stack([res.results[c]["out"] for c in range(N_CORES)])
    return out.astype(np.float32)


# revision 16
# speedup vs baseline: 1.0299x; 1.0299x over previous
"""Trainium2 Bass kernel for nn_DenoiseNet (dense_cnn), 8-core data parallel.

Algorithm (exact, validated vs reference in fp32):
  The kernel-prediction path (conv3x3(3->64) -> depthwise3x3 -> 1x1(64->27))
  is linear in x, so it is folded on the host into ONE composed 5x5 conv
  (3->27 channels) plus:
    - a spatially-uniform term from the ECA channel attention, computed on
      device from cheap reductions of x via a host-folded 27x73 linear map,
    - 1-pixel border-ring corrections computed on device as small matmuls.
  Dynamic filtering = elementwise multiply of the 27 per-pixel kernel maps with
  9 shifted copies of x, then a 0/1 tap-reduction matmul.

Performance structure (vs the previous revision):
  - x is loaded once as fp32 into SBUF; the padded bf16 DRAM staging (xpg) is
    produced by gpsimd cast-DMAs (SWDGE path: no HWDGE contention, casts in
    flight, 3-dim APs).
  - Per band only 5 gpsimd DMAs build the im2col (2) and tap-shift (3) tiles;
    output-channel order is host-permuted to (ty, g, j, tx) so every gather is
    a contiguous/strided partition slice with a <=3-dim AP.
  - Main conv accumulates row PAIRS into 2-bank PSUM tiles; single ACT
    activation drains each pair with the uniform-term bias.
  - Tap matmuls of band b are issued after the main matmuls of band b+1
    (software pipelining) so the PE stream never waits on the DVE multiply;
    PSUM out tiles are drained by DVE and stored by SP.
"""
import numpy as np
import ml_dtypes

import concourse.bass as bass
import concourse.bacc as bacc
import concourse.mybir as mybir
import concourse.tile as tile
from concourse.ap import AP
from concourse.bass_utils import run_bass_kernel_spmd

BF16 = ml_dtypes.bfloat16
H = W = 512
HP, WP = 516, 520
CH = 64
KO = 27
N_CORES = 8
BANDS = 8
BR = 16          # rows per g-chunk per band
CHS = HP * WP
GRS = 128 * WP
BS = 133 * WP    # xpg block stride

F32 = mybir.dt.float32
BF = mybir.dt.bfloat16

XFREE = 4 * 3 * 524  # xf32 free size per partition (elements)


# ------------------------------------------------------------------ host math
def _conv_compose_2d(wa, wb):
    """Compose two cross-correlation kernels (wb after wa).
    wa [C, I, 3, 3], wb [C, 3, 3] depthwise -> [C, I, 5, 5]."""
    C, I = wa.shape[:2]
    out = np.zeros((C, I, 5, 5), wa.dtype)
    for a in range(3):
        for b in range(3):
            out[:, :, a:a + 3, b:b + 3] += wb[:, a, b][:, None, None, None] * wa
    return out


def precompute(w_feat, b_feat, w_sa, b_sa, w_ca, b_ca, w_k, b_k):
    """Fold all weights. float64 internally. Returns dict of np arrays."""
    w_feat = w_feat.astype(np.float64)
    b_feat = b_feat.astype(np.float64)
    w_sa0 = w_sa[:, 0].astype(np.float64)
    b_sa = b_sa.astype(np.float64)
    w_ca = np.asarray(w_ca).astype(np.float64)
    b_ca = float(np.asarray(b_ca).reshape(-1)[0])
    w_k = w_k.astype(np.float64)
    b_k = b_k.astype(np.float64)

    # composed 5x5
    W5 = _conv_compose_2d(w_feat, w_sa0)                 # [C, 3, 5, 5]
    WK5 = np.einsum("oc,cist->oist", w_k, W5)            # [27, 3, 5, 5]
    W1 = w_sa0.sum(axis=(1, 2))
    const_o = w_k @ (b_feat * W1 + b_sa) + b_k           # [27]

    # w5e[pp][k=30g+10i+2dy+dx, m=27g+o] (old m-order; permuted at the end)
    w5e = np.zeros((3, 120, 108))
    for pp in range(3):
        for g in range(4):
            for i in range(3):
                for dy in range(5):
                    for dx in range(2):
                        dxa = 2 * pp + dx
                        if dxa > 4:
                            continue
                        k = 30 * g + 10 * i + 2 * dy + dx
                        w5e[pp, k, 27 * g + np.arange(27)] = WK5[:, i, dy, dxa]
    # sel [108, 12]: row 27g+9j+t -> col 3g+j
    sel = np.zeros((108, 12))
    for g in range(4):
        for j in range(3):
            for t in range(9):
                sel[27 * g + 9 * j + t, 3 * g + j] = 1.0

    # ---- cvec = BV.T @ v (old v layout, re-rowed at the end):
    #  0..11  P1[(s,i)]  12..23 P2L  24..35 P2R  36..47 RS0  48..59 RSL
    #  60..71 corners x[i, 511cy, 511cx] at 60+cy*6+i*2+cx; 72 = 1.0
    MxT = np.zeros((3, 3, 3, 73))
    for i in range(3):
        S = np.zeros(73)
        for s in range(4):
            S[s * 3 + i] = 1.0
        r_ex = {0: np.zeros(73), 1: None, 2: np.zeros(73)}
        r_ex[0][48 + 9 + i] = 1.0     # RSL s=3 -> row 511 excluded for a=0
        r_ex[2][36 + 0 + i] = 1.0     # RS0 s=0 -> row 0 excluded for a=2
        c_ex = {0: np.zeros(73), 1: None, 2: np.zeros(73)}
        for s in range(4):
            c_ex[0][24 + s * 3 + i] = 1.0   # P2R: col 511
            c_ex[2][12 + s * 3 + i] = 1.0   # P2L: col 0
        corner = {(0, 0): (1, 1), (0, 2): (1, 0), (2, 0): (0, 1), (2, 2): (0, 0)}
        for a in range(3):
            for b in range(3):
                m = S.copy()
                if r_ex[a] is not None:
                    m -= r_ex[a]
                if c_ex[b] is not None:
                    m -= c_ex[b]
                if (a, b) in corner:
                    cy, cx = corner[(a, b)]
                    m[60 + cy * 6 + i * 2 + cx] += 1.0
                MxT[i, a, b] = m
    meanT = np.einsum("ciab,iabv->cv", w_feat, MxT) / (H * W)   # [C, 73]
    meanT[:, 72] += b_feat
    caT = np.zeros((CH, 73))
    for d in range(3):
        lo = max(0, 1 - d)
        hi = min(CH, CH + 1 - d)
        caT[lo:hi] += w_ca[0, 0, d] * meanT[d - 1 + lo: d - 1 + hi]
    caT[:, 72] += b_ca
    cvT = w_k @ caT
    cvT[:, 72] += const_o
    BV = np.zeros((73, 108))
    for g in range(4):
        BV[:, 27 * g:27 * g + 27] = cvT.T

    # ---- borders ----
    def wb_1d(sa_row, feat_row):
        out = np.zeros((KO, 3, 5))
        for qx in range(3):
            wk_sa = w_k * sa_row[:, qx][None, :]
            for dx in range(3):
                out[:, :, qx + dx] += wk_sa @ feat_row[:, :, dx]
        return out

    WBtop = wb_1d(w_sa0[:, 0, :], w_feat[:, :, 2, :])
    WBbot = wb_1d(w_sa0[:, 2, :], w_feat[:, :, 0, :])
    WBleft = wb_1d(w_sa0[:, :, 0], w_feat[:, :, :, 2])
    WBright = wb_1d(w_sa0[:, :, 2], w_feat[:, :, :, 0])
    cW = {"t": w_k @ (w_sa0[:, 0, :].sum(1) * b_feat),
          "b": w_k @ (w_sa0[:, 2, :].sum(1) * b_feat),
          "l": w_k @ (w_sa0[:, :, 0].sum(1) * b_feat),
          "r": w_k @ (w_sa0[:, :, 2].sum(1) * b_feat)}

    tw = np.zeros((31, 108))
    bw = np.zeros((31, 108))
    for i in range(3):
        for s in range(5):
            tw[i * 5 + s, 0:27] = WBtop[:, i, s]
            bw[15 + i * 5 + s, 81:108] = WBbot[:, i, s]
    tw[30, 0:27] = cW["t"]
    bw[30, 81:108] = cW["b"]

    # L/R border data maps on im2col partitions k = 30g + 10i + 2s (dx=0
    # slots); the row-independent constant part is applied separately.
    lw120 = np.zeros((120, 108))
    rw120 = np.zeros((120, 108))
    for g in range(4):
        for i in range(3):
            for s in range(5):
                k = 30 * g + 10 * i + 2 * s
                lw120[k, 27 * g:27 * g + 27] = WBleft[:, i, s]
                rw120[k, 27 * g:27 * g + 27] = WBright[:, i, s]
    cwl2 = np.zeros((108, 2))
    cwr2 = np.zeros((108, 2))
    for g in range(4):
        cwl2[27 * g:27 * g + 27, :] = cW["l"][:, None]
        cwr2[27 * g:27 * g + 27, :] = cW["r"][:, None]

    # corner double-count add-backs, split into top (g-block 0) and bottom
    # (g-block 3) variants so edge bands can apply full-width vectors.
    cwl_t = np.zeros((7, 108))
    cwl_b = np.zeros((7, 108))
    cwr_t = np.zeros((7, 108))
    cwr_b = np.zeros((7, 108))
    specs = [
        (cwl_t, 0, 0, (0, 0), (2, 2)),   # TL
        (cwl_b, 1, 3, (2, 0), (0, 2)),   # BL
        (cwr_t, 0, 0, (0, 2), (2, 0)),   # TR
        (cwr_b, 1, 3, (2, 2), (0, 0)),   # BR
    ]
    for M, cy, gblk, (qy, qx), (a, b) in specs:
        wk_sa = w_k * w_sa0[:, qy, qx][None, :]
        A = wk_sa @ w_feat[:, :, a, b]
        c0 = wk_sa @ b_feat
        for i in range(3):
            M[i * 2 + cy, 27 * gblk:27 * gblk + 27] = A[:, i]
        M[6, 27 * gblk:27 * gblk + 27] += c0

    # ---- output-channel permutation: m_new = 36ty + 9g + 3j + tx ----
    perm = np.zeros(108, np.int64)
    for g in range(4):
        for j in range(3):
            for ty in range(3):
                for tx in range(3):
                    m_old = 27 * g + 9 * j + 3 * ty + tx
                    m_new = 36 * ty + 9 * g + 3 * j + tx
                    perm[m_new] = m_old
    w5e = w5e[:, :, perm]
    sel = sel[perm, :]
    tw = -tw[:, perm]
    bw = -bw[:, perm]
    lw120 = -lw120[:, perm]
    rw120 = -rw120[:, perm]
    cwl2 = cwl2[perm, :]
    cwr2 = cwr2[perm, :]
    cwl_t = cwl_t[:, perm]
    cwl_b = cwl_b[:, perm]
    cwr_t = cwr_t[:, perm]
    cwr_b = cwr_b[:, perm]
    BV = BV[:, perm]
    # v-row reorder: first 36 entries interleaved as v_new[3*si + kind]
    BVr = BV.copy()
    for si in range(12):
        for kind in range(3):
            BVr[3 * si + kind] = BV[kind * 12 + si]
    return dict(w5e=w5e, sel=sel, BV=BVr, tw=tw, bw=bw,
                lw120=lw120, rw120=rw120, cwl2=cwl2, cwr2=cwr2,
                cwl_t=cwl_t, cwl_b=cwl_b, cwr_t=cwr_t, cwr_b=cwr_b)


def make_wmaps(pre):
    """Device weight arrays with final dtypes (BV fp32, rest bf16)."""
    out = {}
    for k, v in pre.items():
        if k == "BV":
            out[k] = np.ascontiguousarray(v.astype(np.float32))
        else:
            out[k] = np.ascontiguousarray(v.astype(np.float32).astype(BF16))
    return out


# ------------------------------------------------------------------ device IR
def _h(t):
    """TensorHandle from handle-or-AP."""
    return getattr(t, "tensor", t)


def build_nc():
    nc = bacc.Bacc("TRN2", target_bir_lowering=False, debug=False,
                   num_devices=N_CORES)
    x_ext = nc.declare_dram_parameter("x", (3, H, W), F32, isOutput=False)
    out_ext = nc.declare_dram_parameter("out", (3, H, W), F32, isOutput=True)
    wnames = {"w5e": ((3, 120, 108), BF), "sel": ((108, 12), BF),
              "BV": ((73, 108), F32),
              "tw": ((31, 108), BF), "bw": ((31, 108), BF),
              "lw120": ((120, 108), BF), "rw120": ((120, 108), BF),
              "cwl2": ((108, 2), BF), "cwr2": ((108, 2), BF),
              "cwl_t": ((7, 108), BF), "cwl_b": ((7, 108), BF),
              "cwr_t": ((7, 108), BF), "cwr_b": ((7, 108), BF)}
    wext = {k: nc.declare_dram_parameter(k, shp, dt, isOutput=False)
            for k, (shp, dt) in wnames.items()}
    xpg = nc.dram_tensor("xpg", (12, 133, WP), BF)

    IDENT = mybir.ActivationFunctionType.Identity
    SUB = mybir.AluOpType.subtract
    ADD = mybir.AluOpType.add
    MULT = mybir.AluOpType.mult

    _ring = [0]

    def dma(out, in_):
        """Alternate small prologue DMAs across the two HWDGE engines."""
        _ring[0] ^= 1
        eng = nc.sync if _ring[0] else nc.scalar
        eng.dma_start(out, in_)

    with tile.TileContext(nc) as tc:
        with tc.tile_pool(name="const", bufs=1) as cpool, \
             tc.tile_pool(name="xres", bufs=1) as xpool, \
             tc.tile_pool(name="psPro", bufs=2, space="PSUM") as psP:

            # ---- critical weights, then x loads (HWDGE) ----
            w5e_sb = cpool.tile([120, 3, 108], BF)
            dma(w5e_sb[:], wext["w5e"][:].transpose([1, 0, 2]))
            lw_sb = cpool.tile([120, 108], BF)
            dma(lw_sb[:], wext["lw120"][:])
            rw_sb = cpool.tile([120, 108], BF)
            dma(rw_sb[:], wext["rw120"][:])

            xf = xpool.tile([128, 4, 3, 524], F32)
            pitch = xf[:].ap[0][0]
            xft = _h(xf[:].tensor)
            xoff = xf[:].offset
            nc.vector.memset(xf[:, :, :, 0:2], 0.0)
            nc.vector.memset(xf[:, :, :, 514:524], 0.0)
            for hh in range(2):
                p0 = 64 * hh
                for i in range(3):
                    eng = nc.sync if (i + hh) % 2 else nc.scalar
                    eng.dma_start(
                        xf[p0:p0 + 64, :, i, 2:514],
                        AP(_h(x_ext), i * 262144 + p0 * 512,
                           [[512, 64], [65536, 4], [1, 512]]))

            ztile = cpool.tile([12, 1056], BF)
            nc.vector.memset(ztile[:], 0.0)
            onescol = cpool.tile([128, 1], F32)
            nc.vector.memset(onescol[:], 1.0)
            onesbf = cpool.tile([1, 512], BF)
            nc.vector.memset(onesbf[:], 1.0)

            # ---- xpg zero pads (gpsimd, before staging) ----
            nc.gpsimd.dma_start(AP(_h(xpg), 0, [[BS, 12], [WP, 132], [1, 2]]),
                                ztile[0:12, 0:264])
            nc.gpsimd.dma_start(AP(_h(xpg), 514,
                                   [[BS, 12], [WP, 132], [1, 6]]),
                                ztile[0:12, 0:792])
            nc.gpsimd.dma_start(AP(_h(xpg), 0, [[BS, 3], [1, 2 * WP]]),
                                ztile[0:3, 0:2 * WP])
            nc.gpsimd.dma_start(AP(_h(xpg), 9 * BS + 130 * WP,
                                   [[BS, 3], [1, 2 * WP]]),
                                ztile[0:3, 0:2 * WP])

            # ---- stage xpg (gpsimd cast-DMAs) ----
            for hh in range(2):
                p0 = 64 * hh
                nc.gpsimd.dma_start(
                    AP(_h(xpg), (2 + p0) * WP + 2, [[WP, 64], [BS, 12], [1, 512]]),
                    AP(xft, xoff + p0 * pitch + 2, [[pitch, 64], [524, 12], [1, 512]]))
            # halos: bottom halo of blocks 0..8 <- next chunk rows 0:2
            nc.gpsimd.dma_start(
                AP(_h(xpg), 130 * WP + 2, [[WP, 2], [BS, 9], [1, 512]]),
                AP(xft, xoff + 3 * 524 + 2, [[pitch, 2], [524, 9], [1, 512]]))
            # top halo of blocks 3..11 <- prev chunk rows 126:128
            nc.gpsimd.dma_start(
                AP(_h(xpg), 3 * BS + 2, [[WP, 2], [BS, 9], [1, 512]]),
                AP(xft, xoff + 126 * pitch + 2, [[pitch, 2], [524, 9], [1, 512]]))

            # ---- remaining weights ----
            sel_sb = cpool.tile([108, 12], BF)
            dma(sel_sb[:], wext["sel"][:])
            bv_sb = cpool.tile([73, 108], F32)
            dma(bv_sb[:], wext["BV"][:])
            tw_sb = cpool.tile([31, 108], BF)
            dma(tw_sb[:], wext["tw"][:])
            bw_sb = cpool.tile([31, 108], BF)
            dma(bw_sb[:], wext["bw"][:])
            cwl2_sb = cpool.tile([108, 2], BF)
            dma(cwl2_sb[:], wext["cwl2"][:])
            cwr2_sb = cpool.tile([108, 2], BF)
            dma(cwr2_sb[:], wext["cwr2"][:])
            cw_sb = {}
            for k in ("cwl_t", "cwl_b", "cwr_t", "cwr_b"):
                cw_sb[k] = cpool.tile([7, 108], BF, name=k + "_w")
                dma(cw_sb[k][:], wext[k][:])

            # ---- reductions for the uniform (ca) term ----
            rs = cpool.tile([128, 12], F32)
            nc.vector.reduce_sum(rs[:], xf[:, :, :, 2:514],
                                 axis=mybir.AxisListType.X)
            pvt = psP.tile([128, 512], F32, tag="pro", bufs=2, name="pv_ps")
            nc.tensor.matmul(pvt[0:12, 0:1], lhsT=rs[:], rhs=onescol[:],
                             start=True, stop=True)
            nc.tensor.matmul(pvt[0:12, 1:2], lhsT=xf[:, :, :, 2],
                             rhs=onescol[:], start=True, stop=True)
            nc.tensor.matmul(pvt[0:12, 2:3], lhsT=xf[:, :, :, 513],
                             rhs=onescol[:], start=True, stop=True)
            pv_sb = cpool.tile([12, 3], F32)
            nc.scalar.activation(pv_sb[:], pvt[0:12, 0:3], IDENT)

            vt = cpool.tile([73, 1], F32)
            dma(vt[0:36], pv_sb[:])
            dma(vt[36:48], rs[0:1, :])
            dma(vt[48:60], rs[127:128, :])
            for cy in range(2):
                for cx in range(2):
                    dma(vt[60 + cy * 6 + cx:66 + cy * 6 + cx:2],
                        AP(xft, xoff + cy * 127 * pitch + (9 * cy) * 524 + 2
                           + cx * 511,
                           [[pitch, 1], [524, 3], [1, 1]]))
            dma(vt[72:73], onescol[0:1, 0:1])
            cvp = psP.tile([128, 512], F32, tag="pro", bufs=2, name="cv_ps")
            nc.tensor.matmul(cvp[0:108, 0:1], lhsT=bv_sb[:], rhs=vt[:],
                             start=True, stop=True)
            cvec_sb = cpool.tile([108, 1], F32)
            nc.scalar.activation(cvec_sb[:], cvp[0:108, 0:1], IDENT)

            # ---- border-map rhs gathers ----
            tb_rhs = cpool.tile([31, 512], BF)
            dma(tb_rhs[0:15], AP(_h(xpg), 2 * WP, [[BS, 3], [1, 5], [1, 512]]))
            dma(tb_rhs[15:30],
                AP(_h(xpg), 9 * BS + 129 * WP, [[BS, 3], [1, 5], [1, 512]]))
            dma(tb_rhs[30:31], onesbf[0:1, 0:512])
            crl = cpool.tile([7, 1], BF)
            dma(crl[0:6], AP(_h(xpg), 2 * WP + 2,
                             [[BS, 3], [9 * BS + 127 * WP, 2], [1, 1]]))
            dma(crl[6:7], onesbf[0:1, 0:1])
            crr = cpool.tile([7, 1], BF)
            dma(crr[0:6], AP(_h(xpg), 2 * WP + 513,
                             [[BS, 3], [9 * BS + 127 * WP, 2], [1, 1]]))
            dma(crr[6:7], onesbf[0:1, 0:1])

            # ---- transposed corner maps (accumulated into kps later) ----
            def mm_to_bf(name, lhsT, rhs, pdim, ndim):
                ps = psP.tile([128, 512], F32, tag="pro", bufs=2,
                              name=name + "_ps")
                nc.tensor.matmul(ps[0:pdim, 0:ndim], lhsT=lhsT, rhs=rhs,
                                 start=True, stop=True)
                sb = cpool.tile([pdim, ndim], BF, name=name)
                nc.scalar.activation(sb[:], ps[0:pdim, 0:ndim], IDENT)
                return sb

            c4 = {k: mm_to_bf(k + "T", (crl if "l_" in k else crr)[:],
                              cw_sb[k][:], 1, 108)
                  for k in ("cwl_t", "cwl_b", "cwr_t", "cwr_b")}

            # ---- band loop, software-pipelined taps ----
            bp_ctx = tc.tile_pool(name="band", bufs=1)
            bpool = bp_ctx.__enter__()
            psK_ctx = tc.tile_pool(name="psK", bufs=1, space="PSUM")
            psK = psK_ctx.__enter__()
            psO_ctx = tc.tile_pool(name="psO", bufs=1, space="PSUM")
            psO = psO_ctx.__enter__()

            def emit_taps(pb, prodp):
                r0p = pb * BR
                outb = bpool.tile([76, 8, 512], F32, tag="outsb", bufs=2,
                                  name=f"outsb{pb}")
                for q in range(8):
                    o2 = psO.tile([76, 512], F32, tag="o2", bufs=2,
                                  name=f"o2_{pb}_{q}")
                    nc.tensor.matmul(o2[0:12, :], lhsT=sel_sb[:],
                                     rhs=prodp[:, 2 * q, :],
                                     start=True, stop=True)
                    nc.tensor.matmul(o2[64:76, :], lhsT=sel_sb[:],
                                     rhs=prodp[:, 2 * q + 1, :],
                                     start=True, stop=True)
                    nc.scalar.activation(outb[:, q, :], o2[:], IDENT)
                for j in range(3):
                    nc.sync.dma_start(
                        AP(_h(out_ext), j * 262144 + r0p * 512,
                           [[65536, 4], [1024, 8], [1, 512]]),
                        outb[j:12:3])
                    nc.sync.dma_start(
                        AP(_h(out_ext), j * 262144 + (r0p + 1) * 512,
                           [[65536, 4], [1024, 8], [1, 512]]),
                        outb[64 + j:76:3])

            order = [1, 2, 3, 4, 5, 6, 0, 7]

            def emit_loads(b):
                r0 = b * BR
                imcol = bpool.tile([120, BR, 520], BF, tag="imcol", bufs=3,
                                   name=f"imcol{b}")
                for dx in range(2):
                    eng = nc.sync if dx == 0 else nc.scalar
                    eng.dma_start(
                        imcol[dx::2],
                        AP(_h(xpg), r0 * WP + dx,
                           [[BS, 12], [WP, 5], [1, BR * WP]]))
                xs = bpool.tile([108, BR, 520], BF, tag="xs", bufs=3,
                                name=f"xs{b}")
                for ty in range(3):
                    nc.gpsimd.dma_start(
                        xs[36 * ty:36 * ty + 36],
                        AP(_h(xpg), (r0 + ty + 1) * WP + 1,
                           [[BS, 12], [1, 3], [1, BR * WP]]))
                return imcol, xs

            loads = {order[0]: emit_loads(order[0])}
            prev = None
            for idx, b in enumerate(order):
                r0 = b * BR
                imcol, xs = loads.pop(b)
                if idx + 1 < len(order):
                    loads[order[idx + 1]] = emit_loads(order[idx + 1])

                ks = bpool.tile([108, BR, 512], BF, tag="ks", bufs=2,
                                name=f"ks{b}")
                for pr in range(8):
                    kps = psK.tile([108, 2, 512], F32, tag="kps", bufs=2,
                                   name=f"kps{b}_{pr}")
                    for rr in range(2):
                        r = 2 * pr + rr
                        R = r0 + r
                        chain = [(kps[:, rr, :], w5e_sb[:, 0, :],
                                  imcol[:, r, 0:512]),
                                 (kps[:, rr, :], w5e_sb[:, 1, :],
                                  imcol[:, r, 2:514]),
                                 (kps[:, rr, :], w5e_sb[:, 2, :],
                                  imcol[:, r, 4:516])]
                        if b == 0 and r == 0:
                            chain.append((kps[:, rr, :], tw_sb[:], tb_rhs[:]))
                            chain.append((kps[:, rr, 0:1], c4["cwl_t"][:],
                                          onesbf[0:1, 0:1]))
                            chain.append((kps[:, rr, 511:512], c4["cwr_t"][:],
                                          onesbf[0:1, 0:1]))
                        if b == BANDS - 1 and r == BR - 1:
                            chain.append((kps[:, rr, :], bw_sb[:], tb_rhs[:]))
                            chain.append((kps[:, rr, 0:1], c4["cwl_b"][:],
                                          onesbf[0:1, 0:1]))
                            chain.append((kps[:, rr, 511:512], c4["cwr_b"][:],
                                          onesbf[0:1, 0:1]))
                        chain.append((kps[:, rr, 0:1], lw_sb[:],
                                      imcol[:, r, 2:3]))
                        chain.append((kps[:, rr, 511:512], rw_sb[:],
                                      imcol[:, r, 513:514]))
                        for ci, (dst, lh, rh) in enumerate(chain):
                            nc.tensor.matmul(dst, lhsT=lh, rhs=rh,
                                             start=(ci == 0),
                                             stop=(ci == len(chain) - 1))
                    sl = slice(2 * pr, 2 * pr + 2)
                    nc.scalar.activation(ks[:, sl, :], kps[:], IDENT,
                                         bias=cvec_sb[:])
                    nc.vector.tensor_tensor(ks[:, sl, 0], ks[:, sl, 0],
                                            cwl2_sb[:], op=SUB)
                    nc.vector.tensor_tensor(ks[:, sl, 511], ks[:, sl, 511],
                                            cwr2_sb[:], op=SUB)
                    nc.vector.tensor_tensor(ks[:, sl, :], ks[:, sl, :],
                                            xs[:, sl, 0:512], op=MULT)
                if prev is not None:
                    emit_taps(*prev)
                prev = (b, ks)
            emit_taps(*prev)
            psO_ctx.__exit__(None, None, None)
            psK_ctx.__exit__(None, None, None)
            bp_ctx.__exit__(None, None, None)
    nc.compile()
    return nc


_NC_CACHE = None


def prepare_in_maps(inputs):
    x = np.ascontiguousarray(np.asarray(inputs["x"], np.float32))
    pre = precompute(inputs["w_feat"], inputs["b_feat"], inputs["w_sa"],
                     inputs["b_sa"], inputs["w_ca"], inputs["b_ca"],
                     inputs["w_k"], inputs["b_k"])
    wmaps = make_wmaps(pre)
    return [dict(x=x[c], **wmaps) for c in range(N_CORES)]


def kernel(**inputs):
    global _NC_CACHE
    if _NC_CACHE is None:
        _NC_CACHE = build_nc()
    nc = _NC_CACHE
    in_maps = prepare_in_maps(inputs)
    res = run_bass_kernel_spmd(nc, in_maps, core_ids=list(range(N_CORES)))
    out = np.stack([res.results[c]["out"] for c in range(N_CORES)])
    return out.astype(np.float32)


# revision 28
# speedup vs baseline: 1.1567x; 1.1231x over previous
"""Trainium2 Bass kernel for nn_DenoiseNet (dense_cnn), 8-core data parallel.

Algorithm (exact, validated vs reference in fp32):
  The kernel-prediction path (conv3x3(3->64) -> depthwise3x3 -> 1x1(64->27))
  is linear in x, so it is folded on the host into ONE composed 5x5 conv
  (3->27 channels) plus:
    - a spatially-uniform term from the ECA channel attention, computed on
      device from cheap reductions of x via a host-folded 27x73 linear map,
    - 1-pixel border-ring corrections computed on device as small matmuls.
  Dynamic filtering = elementwise multiply of the 27 per-pixel kernel maps with
  9 shifted copies of x, then a 0/1 tap-reduction matmul.

Performance structure (vs the previous revision):
  - x is loaded once as fp32 into SBUF; the padded bf16 DRAM staging (xpg) is
    produced by gpsimd cast-DMAs (SWDGE path: no HWDGE contention, casts in
    flight, 3-dim APs).
  - Per band only 5 gpsimd DMAs build the im2col (2) and tap-shift (3) tiles;
    output-channel order is host-permuted to (ty, g, j, tx) so every gather is
    a contiguous/strided partition slice with a <=3-dim AP.
  - Main conv accumulates row PAIRS into 2-bank PSUM tiles; single ACT
    activation drains each pair with the uniform-term bias.
  - Tap matmuls of band b are issued after the main matmuls of band b+1
    (software pipelining) so the PE stream never waits on the DVE multiply;
    PSUM out tiles are drained by DVE and stored by SP.
"""
import numpy as np
import ml_dtypes

import concourse.bass as bass
import concourse.bacc as bacc
import concourse.mybir as mybir
import concourse.tile as tile
from concourse.ap import AP
from concourse.bass_utils import run_bass_kernel_spmd

BF16 = ml_dtypes.bfloat16
H = W = 512
HP, WP = 516, 520
CH = 64
KO = 27
N_CORES = 8
BANDS = 8
BR = 16          # rows per g-chunk per band
CHS = HP * WP
GRS = 128 * WP
BS = 133 * WP    # xpg block stride

F32 = mybir.dt.float32
BF = mybir.dt.bfloat16

XFREE = 4 * 3 * 524  # xf32 free size per partition (elements)


# ------------------------------------------------------------------ host math
def _conv_compose_2d(wa, wb):
    """Compose two cross-correlation kernels (wb after wa).
    wa [C, I, 3, 3], wb [C, 3, 3] depthwise -> [C, I, 5, 5]."""
    C, I = wa.shape[:2]
    out = np.zeros((C, I, 5, 5), wa.dtype)
    for a in range(3):
        for b in range(3):
            out[:, :, a:a + 3, b:b + 3] += wb[:, a, b][:, None, None, None] * wa
    return out


def precompute(w_feat, b_feat, w_sa, b_sa, w_ca, b_ca, w_k, b_k):
    """Fold all weights. float64 internally. Returns dict of np arrays."""
    w_feat = w_feat.astype(np.float64)
    b_feat = b_feat.astype(np.float64)
    w_sa0 = w_sa[:, 0].astype(np.float64)
    b_sa = b_sa.astype(np.float64)
    w_ca = np.asarray(w_ca).astype(np.float64)
    b_ca = float(np.asarray(b_ca).reshape(-1)[0])
    w_k = w_k.astype(np.float64)
    b_k = b_k.astype(np.float64)

    # composed 5x5
    W5 = _conv_compose_2d(w_feat, w_sa0)                 # [C, 3, 5, 5]
    WK5 = np.einsum("oc,cist->oist", w_k, W5)            # [27, 3, 5, 5]
    W1 = w_sa0.sum(axis=(1, 2))
    const_o = w_k @ (b_feat * W1 + b_sa) + b_k           # [27]

    # w5e[pp][k=30g+10i+2dy+dx, m=27g+o] (old m-order; permuted at the end)
    w5e = np.zeros((3, 120, 108))
    for pp in range(3):
        for g in range(4):
            for i in range(3):
                for dy in range(5):
                    for dx in range(2):
                        dxa = 2 * pp + dx
                        if dxa > 4:
                            continue
                        k = 30 * g + 10 * i + 2 * dy + dx
                        w5e[pp, k, 27 * g + np.arange(27)] = WK5[:, i, dy, dxa]
    # sel [108, 12]: row 27g+9j+t -> col 3g+j
    sel = np.zeros((108, 12))
    for g in range(4):
        for j in range(3):
            for t in range(9):
                sel[27 * g + 9 * j + t, 3 * g + j] = 1.0

    # ---- cvec = BV.T @ v (old v layout, re-rowed at the end):
    #  0..11  P1[(s,i)]  12..23 P2L  24..35 P2R  36..47 RS0  48..59 RSL
    #  60..71 corners x[i, 511cy, 511cx] at 60+cy*6+i*2+cx; 72 = 1.0
    MxT = np.zeros((3, 3, 3, 73))
    for i in range(3):
        S = np.zeros(73)
        for s in range(4):
            S[s * 3 + i] = 1.0
        r_ex = {0: np.zeros(73), 1: None, 2: np.zeros(73)}
        r_ex[0][48 + 9 + i] = 1.0     # RSL s=3 -> row 511 excluded for a=0
        r_ex[2][36 + 0 + i] = 1.0     # RS0 s=0 -> row 0 excluded for a=2
        c_ex = {0: np.zeros(73), 1: None, 2: np.zeros(73)}
        for s in range(4):
            c_ex[0][24 + s * 3 + i] = 1.0   # P2R: col 511
            c_ex[2][12 + s * 3 + i] = 1.0   # P2L: col 0
        corner = {(0, 0): (1, 1), (0, 2): (1, 0), (2, 0): (0, 1), (2, 2): (0, 0)}
        for a in range(3):
            for b in range(3):
                m = S.copy()
                if r_ex[a] is not None:
                    m -= r_ex[a]
                if c_ex[b] is not None:
                    m -= c_ex[b]
                if (a, b) in corner:
                    cy, cx = corner[(a, b)]
                    m[60 + cy * 6 + i * 2 + cx] += 1.0
                MxT[i, a, b] = m
    meanT = np.einsum("ciab,iabv->cv", w_feat, MxT) / (H * W)   # [C, 73]
    meanT[:, 72] += b_feat
    caT = np.zeros((CH, 73))
    for d in range(3):
        lo = max(0, 1 - d)
        hi = min(CH, CH + 1 - d)
        caT[lo:hi] += w_ca[0, 0, d] * meanT[d - 1 + lo: d - 1 + hi]
    caT[:, 72] += b_ca
    cvT = w_k @ caT
    cvT[:, 72] += const_o
    BV = np.zeros((73, 108))
    for g in range(4):
        BV[:, 27 * g:27 * g + 27] = cvT.T

    # ---- borders ----
    def wb_1d(sa_row, feat_row):
        out = np.zeros((KO, 3, 5))
        for qx in range(3):
            wk_sa = w_k * sa_row[:, qx][None, :]
            for dx in range(3):
                out[:, :, qx + dx] += wk_sa @ feat_row[:, :, dx]
        return out

    WBtop = wb_1d(w_sa0[:, 0, :], w_feat[:, :, 2, :])
    WBbot = wb_1d(w_sa0[:, 2, :], w_feat[:, :, 0, :])
    WBleft = wb_1d(w_sa0[:, :, 0], w_feat[:, :, :, 2])
    WBright = wb_1d(w_sa0[:, :, 2], w_feat[:, :, :, 0])
    cW = {"t": w_k @ (w_sa0[:, 0, :].sum(1) * b_feat),
          "b": w_k @ (w_sa0[:, 2, :].sum(1) * b_feat),
          "l": w_k @ (w_sa0[:, :, 0].sum(1) * b_feat),
          "r": w_k @ (w_sa0[:, :, 2].sum(1) * b_feat)}

    tw = np.zeros((31, 108))
    bw = np.zeros((31, 108))
    for i in range(3):
        for s in range(5):
            tw[i * 5 + s, 0:27] = WBtop[:, i, s]
            bw[15 + i * 5 + s, 81:108] = WBbot[:, i, s]
    tw[30, 0:27] = cW["t"]
    bw[30, 81:108] = cW["b"]

    # L/R border data maps on im2col partitions k = 30g + 10i + 2s (dx=0
    # slots); the row-independent constant part is applied separately.
    lw120 = np.zeros((120, 108))
    rw120 = np.zeros((120, 108))
    for g in range(4):
        for i in range(3):
            for s in range(5):
                k = 30 * g + 10 * i + 2 * s
                lw120[k, 27 * g:27 * g + 27] = WBleft[:, i, s]
                rw120[k, 27 * g:27 * g + 27] = WBright[:, i, s]
    cwl2 = np.zeros((108, 2))
    cwr2 = np.zeros((108, 2))
    for g in range(4):
        cwl2[27 * g:27 * g + 27, :] = cW["l"][:, None]
        cwr2[27 * g:27 * g + 27, :] = cW["r"][:, None]

    # corner double-count add-backs, split into top (g-block 0) and bottom
    # (g-block 3) variants so edge bands can apply full-width vectors.
    cwl_t = np.zeros((7, 108))
    cwl_b = np.zeros((7, 108))
    cwr_t = np.zeros((7, 108))
    cwr_b = np.zeros((7, 108))
    specs = [
        (cwl_t, 0, 0, (0, 0), (2, 2)),   # TL
        (cwl_b, 1, 3, (2, 0), (0, 2)),   # BL
        (cwr_t, 0, 0, (0, 2), (2, 0)),   # TR
        (cwr_b, 1, 3, (2, 2), (0, 0)),   # BR
    ]
    for M, cy, gblk, (qy, qx), (a, b) in specs:
        wk_sa = w_k * w_sa0[:, qy, qx][None, :]
        A = wk_sa @ w_feat[:, :, a, b]
        c0 = wk_sa @ b_feat
        for i in range(3):
            M[i * 2 + cy, 27 * gblk:27 * gblk + 27] = A[:, i]
        M[6, 27 * gblk:27 * gblk + 27] += c0

    # ---- output-channel permutation: m_new = 36ty + 9g + 3j + tx ----
    perm = np.zeros(108, np.int64)
    for g in range(4):
        for j in range(3):
            for ty in range(3):
                for tx in range(3):
                    m_old = 27 * g + 9 * j + 3 * ty + tx
                    m_new = 36 * ty + 9 * g + 3 * j + tx
                    perm[m_new] = m_old
    w5e = w5e[:, :, perm]
    sel = sel[perm, :]
    tw = -tw[:, perm]
    bw = -bw[:, perm]
    lw120 = -lw120[:, perm]
    rw120 = -rw120[:, perm]
    cwl2 = cwl2[perm, :]
    cwr2 = cwr2[perm, :]
    cwl_t = cwl_t[:, perm]
    cwl_b = cwl_b[:, perm]
    cwr_t = cwr_t[:, perm]
    cwr_b = cwr_b[:, perm]
    BV = BV[:, perm]
    # v-row reorder: first 36 entries interleaved as v_new[3*si + kind]
    # Keep only the dominant P1 rows + const of the mean-term map: the
    # row/col/corner exclusion corrections are O(1/512) of the ca term and
    # far below the error budget (~1e-5 relative).
    BV13 = np.zeros((13, 108))
    BV13[0:12] = BV[0:12]
    BV13[12] = BV[72]
    return dict(w5e=w5e, sel=sel, BV=BV13, tw=tw, bw=bw,
                lw120=lw120, rw120=rw120, cwl2=cwl2, cwr2=cwr2,
                cwl_t=cwl_t, cwl_b=cwl_b, cwr_t=cwr_t, cwr_b=cwr_b)


def make_wmaps(pre):
    """Device weight arrays with final dtypes (BV fp32, rest bf16)."""
    out = {}
    for k, v in pre.items():
        if k == "BV":
            out[k] = np.ascontiguousarray(v.astype(np.float32))
        else:
            out[k] = np.ascontiguousarray(v.astype(np.float32).astype(BF16))
    return out


# ------------------------------------------------------------------ device IR
def _h(t):
    """TensorHandle from handle-or-AP."""
    return getattr(t, "tensor", t)


def build_nc():
    nc = bacc.Bacc("TRN2", target_bir_lowering=False, debug=False,
                   num_devices=N_CORES)
    x_ext = nc.declare_dram_parameter("x", (3, H, W), F32, isOutput=False)
    out_ext = nc.declare_dram_parameter("out", (3, H, W), F32, isOutput=True)
    wnames = {"w5e": ((3, 120, 108), BF), "sel": ((108, 12), BF),
              "BV": ((13, 108), F32),
              "tw": ((31, 108), BF), "bw": ((31, 108), BF),
              "lw120": ((120, 108), BF), "rw120": ((120, 108), BF),
              "cwl2": ((108, 2), BF), "cwr2": ((108, 2), BF),
              "cwl_t": ((7, 108), BF), "cwl_b": ((7, 108), BF),
              "cwr_t": ((7, 108), BF), "cwr_b": ((7, 108), BF)}
    wext = {k: nc.declare_dram_parameter(k, shp, dt, isOutput=False)
            for k, (shp, dt) in wnames.items()}
    xpg = nc.dram_tensor("xpg", (12, 133, WP), BF)

    IDENT = mybir.ActivationFunctionType.Identity
    SUB = mybir.AluOpType.subtract
    ADD = mybir.AluOpType.add
    MULT = mybir.AluOpType.mult

    _ring = [0]

    def dma(out, in_):
        """Alternate small prologue DMAs across the two HWDGE engines."""
        _ring[0] ^= 1
        eng = nc.sync if _ring[0] else nc.scalar
        eng.dma_start(out, in_)

    with tile.TileContext(nc) as tc:
        with tc.tile_pool(name="const", bufs=1) as cpool, \
             tc.tile_pool(name="xres", bufs=1) as xpool, \
             tc.tile_pool(name="band", bufs=1) as bpool, \
             tc.tile_pool(name="psPro", bufs=1, space="PSUM") as psP, \
             tc.tile_pool(name="psK", bufs=1, space="PSUM") as psK, \
             tc.tile_pool(name="psO", bufs=1, space="PSUM") as psO:

            # ---- critical weights, then x loads (HWDGE) ----
            w5e_sb = cpool.tile([120, 3, 108], BF)
            dma(w5e_sb[:], wext["w5e"][:].transpose([1, 0, 2]))
            lw_sb = cpool.tile([120, 108], BF)
            dma(lw_sb[:], wext["lw120"][:])
            rw_sb = cpool.tile([120, 108], BF)
            dma(rw_sb[:], wext["rw120"][:])

            xf = xpool.tile([128, 4, 3, 524], BF)
            pitch = xf[:].ap[0][0]
            xft = _h(xf[:].tensor)
            xoff = xf[:].offset
            nc.vector.memset(xf[:, :, :, 0:2], 0.0)
            nc.vector.memset(xf[:, :, :, 514:524], 0.0)
            for hh in range(2):
                p0 = 64 * hh
                for i in range(3):
                    nc.gpsimd.dma_start(
                        xf[p0:p0 + 64, :, i, 2:514],
                        AP(_h(x_ext), i * 262144 + p0 * 512,
                           [[512, 64], [65536, 4], [1, 512]]))

            ztile = cpool.tile([12, 1056], BF)
            nc.vector.memset(ztile[:], 0.0)
            onescol = cpool.tile([128, 1], F32)
            nc.vector.memset(onescol[:], 1.0)
            onescb = cpool.tile([128, 1], BF)
            nc.vector.memset(onescb[:], 1.0)
            onesbf = cpool.tile([1, 512], BF)
            nc.vector.memset(onesbf[:], 1.0)

            # ---- P1 column-sum matmuls (cheap, early cvec) ----
            p1c = psP.tile([128, 512], F32, tag="pro", bufs=1, name="p1c_ps")
            for si in range(12):
                ss, ii = divmod(si, 3)
                for q in range(4):
                    nc.tensor.matmul(
                        p1c[:, si:si + 1],
                        lhsT=xf[:, ss, ii, 2 + 128 * q:130 + 128 * q],
                        rhs=onescb[:], start=(q == 0), stop=(q == 3))
            p1c_sb = cpool.tile([128, 12], F32)
            nc.scalar.activation(p1c_sb[:], p1c[:, 0:12], IDENT)
            vt = cpool.tile([13, 1], F32)
            pvt = psP.tile([128, 512], F32, tag="pro", bufs=1, name="pv_ps")
            nc.tensor.matmul(pvt[0:12, 0:1], lhsT=p1c_sb[:], rhs=onescol[:],
                             start=True, stop=True)
            nc.scalar.activation(vt[0:12], pvt[0:12, 0:1], IDENT)
            dma(vt[12:13], onescol[0:1, 0:1])

            # ---- stage xpg lower half incl. col pads (plain bf16 copy) ----
            dma(AP(_h(xpg), 2 * WP, [[WP, 64], [BS, 12], [1, 520]]),
                AP(xft, xoff, [[pitch, 64], [524, 12], [1, 520]]))

            # ---- band 1+2 loads, ahead of everything non-critical ----
            def emit_loads(b):
                r0 = b * BR
                imcol = bpool.tile([120, BR, 520], BF, tag="imcol", bufs=3,
                                   name=f"imcol{b}")
                for dx in range(2):
                    eng = nc.sync if dx == 0 else nc.scalar
                    eng.dma_start(
                        imcol[dx::2],
                        AP(_h(xpg), r0 * WP + dx,
                           [[BS, 12], [WP, 5], [1, BR * WP]]))
                xs = bpool.tile([108, BR, 520], BF, tag="xs", bufs=3,
                                name=f"xs{b}")
                for ty in range(3):
                    nc.gpsimd.dma_start(
                        xs[36 * ty:36 * ty + 36],
                        AP(_h(xpg), (r0 + ty + 1) * WP + 1,
                           [[BS, 12], [1, 3], [1, BR * WP]]))
                return imcol, xs

            loads = {1: emit_loads(1)}

            # ---- stage xpg upper half + halos + row pads (plain bf16) ----
            dma(AP(_h(xpg), 66 * WP, [[WP, 64], [BS, 12], [1, 520]]),
                AP(xft, xoff + 64 * pitch,
                   [[pitch, 64], [524, 12], [1, 520]]))
            loads[2] = emit_loads(2)
            # halos: bottom halo of blocks 0..8 <- next chunk rows 0:2
            dma(AP(_h(xpg), 130 * WP, [[WP, 2], [BS, 9], [1, 520]]),
                AP(xft, xoff + 3 * 524, [[pitch, 2], [524, 9], [1, 520]]))
            # top halo of blocks 3..11 <- prev chunk rows 126:128
            dma(AP(_h(xpg), 3 * BS, [[WP, 2], [BS, 9], [1, 520]]),
                AP(xft, xoff + 126 * pitch, [[pitch, 2], [524, 9], [1, 520]]))
            dma(AP(_h(xpg), 0, [[BS, 3], [1, 2 * WP]]), ztile[0:3, 0:2 * WP])
            dma(AP(_h(xpg), 9 * BS + 130 * WP, [[BS, 3], [1, 2 * WP]]),
                ztile[0:3, 0:2 * WP])

            # ---- remaining weights ----
            sel_sb = cpool.tile([108, 12], BF)
            dma(sel_sb[:], wext["sel"][:])
            bv_sb = cpool.tile([13, 108], F32)
            dma(bv_sb[:], wext["BV"][:])
            tw_sb = cpool.tile([31, 108], BF)
            dma(tw_sb[:], wext["tw"][:])
            bw_sb = cpool.tile([31, 108], BF)
            dma(bw_sb[:], wext["bw"][:])
            cwl2_sb = cpool.tile([108, 2], BF)
            dma(cwl2_sb[:], wext["cwl2"][:])
            cwr2_sb = cpool.tile([108, 2], BF)
            dma(cwr2_sb[:], wext["cwr2"][:])
            cw_sb = {}
            for k in ("cwl_t", "cwl_b", "cwr_t", "cwr_b"):
                cw_sb[k] = cpool.tile([7, 108], BF, name=k + "_w")
                dma(cw_sb[k][:], wext[k][:])

            # ---- reductions for the uniform (ca) term ----
            rs = cpool.tile([128, 12], F32)
            nc.vector.reduce_sum(rs[:], xf[:, :, :, 2:514],
                                 axis=mybir.AxisListType.X)
            pvt = psP.tile([128, 512], F32, tag="pro", bufs=1, name="pv_ps")
            nc.tensor.matmul(pvt[0:12, 0:1], lhsT=rs[:], rhs=onescol[:],
                             start=True, stop=True)
            nc.tensor.matmul(pvt[0:12, 1:2], lhsT=xf[:, :, :, 2],
                             rhs=onescb[:], start=True, stop=True)
            nc.tensor.matmul(pvt[0:12, 2:3], lhsT=xf[:, :, :, 513],
                             rhs=onescb[:], start=True, stop=True)
            pv_sb = cpool.tile([12, 3], F32)
            nc.scalar.activation(pv_sb[:], pvt[0:12, 0:3], IDENT)

            vt = cpool.tile([73, 1], F32)
            dma(vt[0:36], pv_sb[:])
            dma(vt[36:48], rs[0:1, :])
            dma(vt[48:60], rs[127:128, :])
            for cy in range(2):
                for cx in range(2):
                    nc.gpsimd.dma_start(
                        vt[60 + cy * 6 + cx:66 + cy * 6 + cx:2],
                        AP(xft, xoff + cy * 127 * pitch + (9 * cy) * 524 + 2
                           + cx * 511,
                           [[pitch, 1], [524, 3], [1, 1]]))
            dma(vt[72:73], onescol[0:1, 0:1])
            cvp = psP.tile([128, 512], F32, tag="pro", bufs=1, name="cv_ps")
            nc.tensor.matmul(cvp[0:108, 0:1], lhsT=bv_sb[:], rhs=vt[:],
                             start=True, stop=True)
            cvec_sb = cpool.tile([108, 1], F32)
            nc.scalar.activation(cvec_sb[:], cvp[0:108, 0:1], IDENT)

            # ---- border-map rhs gathers ----
            tb_rhs = cpool.tile([31, 512], BF)
            dma(tb_rhs[0:15], AP(_h(xpg), 2 * WP, [[BS, 3], [1, 5], [1, 512]]))
            dma(tb_rhs[15:30],
                AP(_h(xpg), 9 * BS + 129 * WP, [[BS, 3], [1, 5], [1, 512]]))
            dma(tb_rhs[30:31], onesbf[0:1, 0:512])
            crl = cpool.tile([7, 1], BF)
            dma(crl[0:6], AP(_h(xpg), 2 * WP + 2,
                             [[BS, 3], [9 * BS + 127 * WP, 2], [1, 1]]))
            dma(crl[6:7], onesbf[0:1, 0:1])
            crr = cpool.tile([7, 1], BF)
            dma(crr[0:6], AP(_h(xpg), 2 * WP + 513,
                             [[BS, 3], [9 * BS + 127 * WP, 2], [1, 1]]))
            dma(crr[6:7], onesbf[0:1, 0:1])

            # ---- transposed corner maps (accumulated into kps later) ----
            def mm_to_bf(name, lhsT, rhs, pdim, ndim):
                ps = psP.tile([128, 512], F32, tag="pro", bufs=1,
                              name=name + "_ps")
                nc.tensor.matmul(ps[0:pdim, 0:ndim], lhsT=lhsT, rhs=rhs,
                                 start=True, stop=True)
                sb = cpool.tile([pdim, ndim], BF, name=name)
                nc.scalar.activation(sb[:], ps[0:pdim, 0:ndim], IDENT)
                return sb

            c4 = {k: mm_to_bf(k + "T", (crl if "l_" in k else crr)[:],
                              cw_sb[k][:], 1, 108)
                  for k in ("cwl_t", "cwl_b", "cwr_t", "cwr_b")}

            # ---- band loop, software-pipelined taps ----
            def emit_taps(pb, prodp):
                r0p = pb * BR
                outb = bpool.tile([76, 8, 512], F32, tag="outsb", bufs=2,
                                  name=f"outsb{pb}")
                for q in range(8):
                    o2 = psO.tile([76, 512], F32, tag="o2", bufs=3,
                                  name=f"o2_{pb}_{q}")
                    nc.tensor.matmul(o2[0:12, :], lhsT=sel_sb[:],
                                     rhs=prodp[:, 2 * q, :],
                                     start=True, stop=True)
                    nc.tensor.matmul(o2[64:76, :], lhsT=sel_sb[:],
                                     rhs=prodp[:, 2 * q + 1, :],
                                     start=True, stop=True)
                    nc.scalar.activation(outb[:, q, :], o2[:], IDENT)
                for j in range(3):
                    nc.gpsimd.dma_start(
                        AP(_h(out_ext), j * 262144 + r0p * 512,
                           [[65536, 4], [1024, 8], [1, 512]]),
                        outb[j:12:3])
                    nc.gpsimd.dma_start(
                        AP(_h(out_ext), j * 262144 + (r0p + 1) * 512,
                           [[65536, 4], [1024, 8], [1, 512]]),
                        outb[64 + j:76:3])

            order = [1, 2, 3, 4, 5, 6, 0, 7]
            prev = None
            for idx, b in enumerate(order):
                r0 = b * BR
                imcol, xs = loads.pop(b)
                if idx + 2 < len(order):
                    loads[order[idx + 2]] = emit_loads(order[idx + 2])

                ks = bpool.tile([108, BR, 512], BF, tag="ks", bufs=2,
                                name=f"ks{b}")
                for pr in range(8):
                    kps = psK.tile([108, 2, 512], F32, tag="kps", bufs=2,
                                   name=f"kps{b}_{pr}")
                    for rr in range(2):
                        r = 2 * pr + rr
                        R = r0 + r
                        chain = [(kps[:, rr, :], w5e_sb[:, 0, :],
                                  imcol[:, r, 0:512]),
                                 (kps[:, rr, :], w5e_sb[:, 1, :],
                                  imcol[:, r, 2:514]),
                                 (kps[:, rr, :], w5e_sb[:, 2, :],
                                  imcol[:, r, 4:516])]
                        if b == 0 and r == 0:
                            chain.append((kps[:, rr, :], tw_sb[:], tb_rhs[:]))
                            chain.append((kps[:, rr, 0:1], c4["cwl_t"][:],
                                          onesbf[0:1, 0:1]))
                            chain.append((kps[:, rr, 511:512], c4["cwr_t"][:],
                                          onesbf[0:1, 0:1]))
                        if b == BANDS - 1 and r == BR - 1:
                            chain.append((kps[:, rr, :], bw_sb[:], tb_rhs[:]))
                            chain.append((kps[:, rr, 0:1], c4["cwl_b"][:],
                                          onesbf[0:1, 0:1]))
                            chain.append((kps[:, rr, 511:512], c4["cwr_b"][:],
                                          onesbf[0:1, 0:1]))
                        chain.append((kps[:, rr, 0:1], lw_sb[:],
                                      imcol[:, r, 2:3]))
                        chain.append((kps[:, rr, 511:512], rw_sb[:],
                                      imcol[:, r, 513:514]))
                        for ci, (dst, lh, rh) in enumerate(chain):
                            nc.tensor.matmul(dst, lhsT=lh, rhs=rh,
                                             start=(ci == 0),
                                             stop=(ci == len(chain) - 1))
                    sl = slice(2 * pr, 2 * pr + 2)
                    nc.scalar.activation(ks[:, sl, :], kps[:], IDENT,
                                         bias=cvec_sb[:])
                    nc.vector.tensor_tensor(ks[:, sl, 0], ks[:, sl, 0],
                                            cwl2_sb[:], op=SUB)
                    nc.vector.tensor_tensor(ks[:, sl, 511], ks[:, sl, 511],
                                            cwr2_sb[:], op=SUB)
                    nc.vector.tensor_tensor(ks[:, sl, :], ks[:, sl, :],
                                            xs[:, sl, 0:512], op=MULT)
                if prev is not None:
                    emit_taps(*prev)
                prev = (b, ks)
            emit_taps(*prev)
    nc.compile()
    return nc


_NC_CACHE = None


def prepare_in_maps(inputs):
    x = np.ascontiguousarray(np.asarray(inputs["x"], np.float32))
    pre = precompute(inputs["w_feat"], inputs["b_feat"], inputs["w_sa"],
                     inputs["b_sa"], inputs["w_ca"], inputs["b_ca"],
                     inputs["w_k"], inputs["b_k"])
    wmaps = make_wmaps(pre)
    return [dict(x=x[c], **wmaps) for c in range(N_CORES)]


def kernel(**inputs):
    global _NC_CACHE
    if _NC_CACHE is None:
        _NC_CACHE = build_nc()
    nc = _NC_CACHE
    in_maps = prepare_in_maps(inputs)
    res = run_bass_kernel_spmd(nc, in_maps, core_ids=list(range(N_CORES)))
    out = np.stack([res.results[c]["out"] for c in range(N_CORES)])
    return out.astype(np.float32)
